# revision 73
# baseline (speedup 1.0000x reference)
"""MlpAttentionLayer Trainium2 kernel.

Math (reference):
  cat = [x, x-q, q]                         [B,T,3D]
  h   = BN1(cat); p = relu(h @ W1)          [B,T,D]
  g   = BN2(p);   w = sigmoid(g @ W2)       [B,T,1]
  out = sum_t x * w                         [B,D]

BN1 is affine per-feature, so with s1 = g1/sqrt(v1+eps):
  p_pre = x @ Wx + q @ Wq + bias0
    Wx    = s1a*W1a + s1b*W1b           (per-row scaled, [D,D])
    Wq    = s1c*W1c - s1b*W1b           ([D,D])
    bias0 = (b1 - m1*s1) @ W1            ([D])
BN2+W2 fold to:  logits = relu(p_pre) @ W2p + c2,  W2p = s2*W2, c2 scalar.

Qp = q @ Wq + bias0 is per-(b, dout) constant over t.  Two exact forms:
  relu(p_pre) = relu(x@Wx + Qp)              (ACT: bias activation)
              = max(x@Wx, -Qp) + Qp          (DVE: broadcast max; the
                                              W2p·Qp lands in the bias)
Half of each group's batches ride each engine; the matching per-b bias
(c2 or c2 + W2p·Qp) is accumulated into the logit PSUM by a K=1 ones
matmul, so the sigmoid needs no bias at all.

Logits are computed TRANSPOSED: lhsT = h1 slice [dout, t<=112] as the
stationary, rhs = W2p column -> logitsT [t-part, 1] per (b, t-half).
That bears the [t-part, b] weight layout for the final matvecs directly:
no 200-col logit streams, no wT transposes, no w-extract copies, and the
sigmoid shrinks to free-size 8.

Device per core (256 batch, groups of 4), software-pipelined slots:
slot s runs cast(s+1) [Pool], xT transposes(s) [PE] + PSUM->SBUF copy
[DVE], pre GEMMs(s-1) [PE] + h1(s-1) [DVE max / ACT relu], logitsT +
bias matvecs(s-2) [PE] + sigmoidT(s-2) [ACT], finals(s-4) [PE].
x loads [t-part, b, d] split a/b, spread over the SP/ACT/Pool queues.
"""

import sys

sys.path.insert(0, "/opt/trn_rl_repo")

import numpy as np
import ml_dtypes

BN_EPS = 1e-3
B, T, D = 2048, 200, 128
N_CORES = 8
BSH = B // N_CORES          # 256 batch elements per core
G = 4                       # batch elements per pipeline group
NGRP = BSH // G             # 64 groups
LB = 4                      # batch elements per DMA load
NLOAD = BSH // LB           # load pairs
GPL = LB // G               # groups per load
PF = 3                      # load prefetch distance (iterations)
TA, TBV = 112, 88           # t-tile split 200 = 112 + 88
TP = TA + TBV               # 200 real tokens
XT_STRIDE = 224             # xT free stride per b (32B-aligned chunks)

BF16 = ml_dtypes.bfloat16
DEBUG = False


def _build_bass():
    from concourse import bacc, mybir
    from concourse.tile import TileContext
    from concourse.masks import make_identity

    fp32 = mybir.dt.float32
    bf16 = mybir.dt.bfloat16
    AF = mybir.ActivationFunctionType
    ALU = mybir.AluOpType

    nc = bacc.Bacc()
    x_d = nc.dram_tensor("x", (BSH, T, D), fp32, kind="ExternalInput")
    qpm_d = nc.dram_tensor("qpm", (D, BSH), fp32, kind="ExternalInput")
    c2s_d = nc.dram_tensor("c2s", (1, BSH), fp32, kind="ExternalInput")
    wx_d = nc.dram_tensor("wx", (D, D), bf16, kind="ExternalInput")
    w2c_d = nc.dram_tensor("w2c", (D, 1), bf16, kind="ExternalInput")
    out_d = nc.dram_tensor("out", (BSH, D), fp32, kind="ExternalOutput")

    with TileContext(nc) as tc:
        with (
            tc.tile_pool(name="const", bufs=1) as cpool,
            tc.tile_pool(name="xina", bufs=12) as xapool,
            tc.tile_pool(name="xinb", bufs=8) as xbpool,
            tc.tile_pool(name="x16", bufs=12) as x16pool,
            tc.tile_pool(name="xt", bufs=3) as xtpool,
            tc.tile_pool(name="mid", bufs=3) as midpool,
            tc.tile_pool(name="wts", bufs=3) as wtpool,
            tc.tile_pool(name="fin", bufs=1) as finpool,
            tc.tile_pool(name="ps_pre", bufs=3, space="PSUM") as pre_pool,
            tc.tile_pool(name="ps_xt", bufs=2, space="PSUM") as xt_pool,
            tc.tile_pool(name="ps_lw", bufs=2, space="PSUM") as lw_pool,
            tc.tile_pool(name="ps_out", bufs=1, space="PSUM") as fout_pool,
        ):
            fout = fout_pool.tile([128, BSH], mybir.dt.float32)

            # b-load queue pattern, period 8: SP x3, ACT x3, Pool x2
            B_ENG = ["sync", "scalar", "gpsimd", "sync", "scalar",
                     "gpsimd", "sync", "scalar"]

            def emit_load_a(li):
                # [t-part, b, d] fp32 loads; t-half a from the SP queue
                lb0 = li * LB
                x32a = xapool.tile([TA, LB, D], fp32, tag="x32a")
                nc.sync.dma_start(
                    x32a, x_d[lb0 : lb0 + LB, 0:TA, :].rearrange("b t d -> t b d")
                )
                return x32a

            def emit_load_b(li):
                # t-half b spread across SP/ACT/Pool queues; emitted
                # mid-slot so it never head-of-line blocks compute already
                # waiting in those queues
                lb0 = li * LB
                x32b = xbpool.tile([TBV, LB, D], fp32, tag="x32b")
                # warmup loads go to the still-idle ACT/Pool queues so the
                # first groups' data lands as early as possible; the last
                # few ride SP, whose queue drains before the compute queues
                if li < PF:
                    name = ["scalar", "gpsimd"][li % 2]
                elif li >= NLOAD - 4:
                    name = "sync"
                else:
                    name = B_ENG[li % 8]
                eng = getattr(nc, name)
                eng.dma_start(
                    x32b, x_d[lb0 : lb0 + LB, TA:T, :].rearrange("b t d -> t b d")
                )
                return x32b

            def emit_final(fin):
                # final matvecs: out_b = x_b^T @ w_b, column-packed PSUM
                x16a, x16b, wtab, cb, b0 = fin
                for g in range(G):
                    bc = b0 + g
                    ca = cb + 2 * g
                    nc.tensor.matmul(
                        fout[:, bc : bc + 1],
                        x16a[:, g, :],
                        wtab[0:TA, ca : ca + 1],
                        start=True,
                        stop=False,
                    )
                    nc.tensor.matmul(
                        fout[:, bc : bc + 1],
                        x16b[:, g, :],
                        wtab[0:TBV, ca + 1 : ca + 2],
                        start=False,
                        stop=True,
                    )

            st_x16 = {}   # gi -> (x16a, x16b)       cast, awaiting transposes
            st_xt = {}    # gi -> (xt, x16a, x16b)   transposed, awaiting pre
            st_h1 = {}    # gi -> (h1, x16a, x16b)   awaiting logitsT
            pending = []  # finals
            la = {li: emit_load_a(li) for li in range(PFA)}
            lb = {li: emit_load_b(li) for li in range(PF)}
            ident16 = cpool.tile([128, 128], bf16)
            make_identity(nc, ident16)
            ident32 = cpool.tile([128, 128], fp32)
            make_identity(nc, ident32)
            wx_sb = cpool.tile([D, D], bf16)
            nc.gpsimd.dma_start(wx_sb, wx_d[:, :])
            w2c_sb = cpool.tile([D, 1], bf16)
            nc.gpsimd.dma_start(w2c_sb, w2c_d[:, :])
            qpm_sb = cpool.tile([128, BSH], fp32)
            nc.gpsimd.dma_start(qpm_sb, qpm_d[:, :])
            c2s_sb = cpool.tile([1, BSH], fp32)
            nc.gpsimd.dma_start(c2s_sb, c2s_d[:, :])
            ones32 = cpool.tile([1, TA], fp32)
            nc.vector.memset(ones32, 1.0)

            def stage0(gi):
                # cast to bf16 one slot ahead of the transposes (Pool)
                li, gl = gi // GPL, gi % GPL
                x32a, x32b = la[li], lb[li]
                x16a = x16pool.tile([TA, G, D], bf16, tag="x16a")
                nc.gpsimd.tensor_copy(
                    x16a.rearrange("p b d -> p (b d)"),
                    x32a[:, G * gl : G * gl + G, :].rearrange("p b d -> p (b d)"),
                )
                x16b = x16pool.tile([TBV, G, D], bf16, tag="x16b")
                nc.gpsimd.tensor_copy(
                    x16b.rearrange("p b d -> p (b d)"),
                    x32b[:, G * gl : G * gl + G, :].rearrange("p b d -> p (b d)"),
                )
                st_x16[gi] = (x16a, x16b)

            def stage1(gi):
                # xT via PE transposes (bf16) into one PSUM bank, then one
                # strided DVE copy to SBUF
                x16a, x16b = st_x16.pop(gi)
                xt = xtpool.tile([128, G, XT_STRIDE], bf16, tag="xt")
                xtp = xt_pool.tile([128, G, XT_STRIDE], bf16, tag="xtp")
                for g in range(G):
                    nc.tensor.transpose(
                        xtp[:, g, 0:TA], x16a[:, g, :], ident16[0:TA, 0:TA]
                    )
                    nc.tensor.transpose(
                        xtp[:, g, TA:TP], x16b[:, g, :], ident16[0:TBV, 0:TBV]
                    )
                nc.vector.tensor_copy(xt[:, :, 0:TP], xtp[:, :, 0:TP])
                st_xt[gi] = (xt, x16a, x16b)

            def stage2(gi):
                # pre GEMMs (PE); h1 for b0,b1 via DVE max(pre, -Qp),
                # for b2,b3 via ACT relu(pre + Qp) bias activations
                xt, x16a, x16b = st_xt.pop(gi)
                b0 = gi * G
                pre0 = pre_pool.tile([128, 2, TP], mybir.dt.float32, tag="pre")
                pre1 = pre_pool.tile([128, 2, TP], mybir.dt.float32, tag="pre")
                nc.tensor.matmul(
                    pre0.rearrange("p b t -> p (b t)"), wx_sb,
                    xt[:, 0:2, 0:TP], start=True, stop=True,
                )
                nc.tensor.matmul(
                    pre1.rearrange("p b t -> p (b t)"), wx_sb,
                    xt[:, 2:4, 0:TP], start=True, stop=True,
                )
                h1 = midpool.tile([128, G, XT_STRIDE], bf16, tag="h1")
                nc.vector.tensor_tensor(
                    h1[:, 0:2, 0:TP], pre0[:, :, :],
                    qpm_sb[:, b0 : b0 + 2].broadcast_to((128, 2, TP)), ALU.max,
                )
                nc.scalar.activation(
                    h1[:, 2, 0:TP], pre1[:, 0, :], AF.Relu,
                    bias=qpm_sb[:, b0 + 2 : b0 + 3],
                )
                nc.scalar.activation(
                    h1[:, 3, 0:TP], pre1[:, 1, :], AF.Relu,
                    bias=qpm_sb[:, b0 + 3 : b0 + 4],
                )
                st_h1[gi] = (h1, x16a, x16b)

            SW = 4        # sigmoid window: groups batched into one ACT op
            win = {}      # rotating (lwt, wtab, stash) for the open window

            def stage3(gi):
                # transposed logits: per (b, t-half) a K=1 ones x c2 matvec
                # seeds the bias, then lhsT = h1 slice vs W2p column gives
                # logitsT [t-part, 1].  Four groups share one lwt bank and
                # one [112, 32] sigmoid (the ACT access latency dominates
                # tiny activations, so batching 4x saves ~140 ns/group).
                h1, x16a, x16b = st_h1.pop(gi)
                b0 = gi * G
                wi = gi % SW
                if wi == 0:
                    lwt = lw_pool.tile([TA, 2 * G * SW], mybir.dt.float32, tag="lwt")
                    wtab = wtpool.tile([TA, 2 * G * SW], bf16, tag="wtab")
                    win.update(lwt=lwt, wtab=wtab, stash=[], fstart=0)
                lwt, wtab = win["lwt"], win["wtab"]
                cb = 2 * G * wi
                for g in range(G):
                    bc = b0 + g
                    for half in range(2):
                        col = cb + 2 * g + half
                        nc.tensor.matmul(
                            lwt[:, col : col + 1],
                            ones32,
                            c2s_sb[:, bc : bc + 1],
                            start=True,
                            stop=False,
                            skip_group_check=True,
                        )
                        hs = h1[:, g, 0:TA] if half == 0 else h1[:, g, TA:TP]
                        mrows = TA if half == 0 else TBV
                        nc.tensor.matmul(
                            lwt[0:mrows, col : col + 1],
                            hs,
                            w2c_sb,
                            start=False,
                            stop=True,
                            skip_group_check=True,
                        )
                win["stash"].append((x16a, x16b, wtab, cb, b0))
                # flush per-window; near the drain flush every group so the
                # tail finals never wait on later groups' logits
                if wi == SW - 1 or gi == NGRP - 1:
                    fs = win["fstart"]
                    nc.scalar.activation(
                        wtab[:, fs : cb + 2 * G], lwt[:, fs : cb + 2 * G],
                        AF.Sigmoid,
                    )
                    win["fstart"] = cb + 2 * G
                    pending.extend(win["stash"])
                    win["stash"] = []

            osb = finpool.tile([128, BSH], mybir.dt.float32)
            obt = finpool.tile([128, BSH], mybir.dt.float32)

            def emit_epilogue(c0, w):
                # transpose accumulated [d, b] -> [b, d] and store; early
                # pieces fire mid-loop as soon as their batch columns are
                # final, so only the last quarter sits in the drain
                nc.vector.tensor_copy(osb[:, c0 : c0 + w], fout[:, c0 : c0 + w])
                ot = pre_pool.tile([128, 2, TP], mybir.dt.float32, tag="pre")
                otv = ot.rearrange("p b t -> p (b t)")
                nc.tensor.transpose(otv[0:w, 0:128], osb[:, c0 : c0 + w], ident32)
                nc.vector.tensor_copy(obt[0:w, 0:128], otv[0:w, 0:128])
                nc.gpsimd.dma_start(out_d[c0 : c0 + w, :], obt[0:w, 0:128])

            n_fin = 0
            for slot in range(NGRP + 2):
                if slot < NGRP:
                    li, gl = slot // GPL, slot % GPL
                    if gl == 0 and li + PFA < NLOAD:
                        la[li + PFA] = emit_load_a(li + PFA)
                if slot == 0:
                    stage0(0)
                    stage0(1)
                elif slot + 1 < NGRP:
                    stage0(slot + 1)
                if slot < NGRP:
                    stage1(slot)
                if 1 <= slot <= NGRP:
                    stage2(slot - 1)
                if 2 <= slot <= NGRP + 1:
                    stage3(slot - 2)
                while len(pending) >= 2:
                    emit_final(pending.pop(0))
                    n_fin += 1
                    if n_fin == 32:
                        emit_epilogue(0, 128)
                if slot < NGRP and slot % GPL == 0 and slot // GPL + PF < NLOAD:
                    lb[slot // GPL + PF] = emit_load_b(slot // GPL + PF)

            for fin in pending:
                emit_final(fin)
            emit_epilogue(128, 128)
    nc.finalize()
    return nc


_NC_CACHE = {}


def _get_nc():
    if "nc" not in _NC_CACHE:
        _NC_CACHE["nc"] = _build_bass()
    return _NC_CACHE["nc"]


def _host_prep(inputs, query, W1, W2, bn1_gamma, bn1_beta, bn1_mean, bn1_var,
               bn2_gamma, bn2_beta, bn2_mean, bn2_var):
    x = np.asarray(inputs, np.float32)
    q = np.asarray(query, np.float64)
    W1 = np.asarray(W1, np.float64)
    W2 = np.asarray(W2, np.float64)
    s1 = np.asarray(bn1_gamma, np.float64) / np.sqrt(
        np.asarray(bn1_var, np.float64) + BN_EPS
    )
    W1s = s1[:, None] * W1                       # scale rows of W1
    Wx = W1s[0:D] + W1s[D : 2 * D]               # [D, D]
    Wq = W1s[2 * D : 3 * D] - W1s[D : 2 * D]     # [D, D]
    bias0 = (np.asarray(bn1_beta, np.float64) - np.asarray(bn1_mean, np.float64) * s1) @ W1
    Qp = q @ Wq + bias0                          # [B, D]
    s2 = np.asarray(bn2_gamma, np.float64) / np.sqrt(
        np.asarray(bn2_var, np.float64) + BN_EPS
    )
    W2p = s2 * W2[:, 0]                          # [D]
    c2 = float(
        (np.asarray(bn2_beta, np.float64) - np.asarray(bn2_mean, np.float64) * s2)
        @ W2[:, 0]
    )
    c2b = c2 + Qp @ W2p                          # [B]

    wx16 = Wx.astype(BF16)                       # lhsT [K=din, M=dout]
    w2c16 = W2p.astype(BF16)[:, None]            # [D, 1]
    # qpm: column b holds -Qp[b] for the DVE max batches (b%4 in {0,1})
    # and +Qp[b] for the ACT relu-bias batches (b%4 in {2,3})
    sign = np.where(np.arange(B) % G < 2, -1.0, 1.0)
    qpm = np.ascontiguousarray((sign[:, None] * Qp).T.astype(np.float32))
    # matching logit bias: c2 + W2p.Qp for max batches, plain c2 for relu
    c2s = np.where(np.arange(B) % G < 2, c2b, c2).astype(np.float32)[None, :]
    return x, wx16, w2c16, qpm, c2s


def kernel(
    inputs,
    query,
    W1,
    W2,
    bn1_gamma,
    bn1_beta,
    bn1_mean,
    bn1_var,
    bn2_gamma,
    bn2_beta,
    bn2_mean,
    bn2_var,
):
    from concourse.bass_utils import run_bass_kernel_spmd

    x, wx16, w2c16, qpm, c2s = _host_prep(
        inputs, query, W1, W2, bn1_gamma, bn1_beta, bn1_mean, bn1_var,
        bn2_gamma, bn2_beta, bn2_mean, bn2_var,
    )

    nc = _get_nc()
    in_maps = []
    for c in range(N_CORES):
        bsl = slice(c * BSH, (c + 1) * BSH)
        in_maps.append(
            {
                "x": x[bsl],
                "qpm": np.ascontiguousarray(qpm[:, bsl]),
                "c2s": np.ascontiguousarray(c2s[:, bsl]),
                "wx": wx16,
                "w2c": w2c16,
            }
        )
    res = run_bass_kernel_spmd(nc, in_maps, core_ids=list(range(N_CORES)))
    out = np.concatenate([r["out"] for r in res.results], axis=0)
    return out.astype(np.float32)
